# revision 15
# baseline (speedup 1.0000x reference)
"""LoRA attention kernel for Trainium2, batch-sharded across 8 NeuronCores.

Strategy (v7):
  - Data parallel: batch B=8 -> one batch element per core.
  - LoRA factors folded into Wqkv on the host (exact algebra, float64);
    the 1/sqrt(hd) score scale is folded into Wq as well.
  - All matmul operands are bfloat16 (1 cycle/row on the PE regardless of
    moving free dim, and FWL halves LDWEIGHTS time); accumulation is fp32
    in PSUM so only operand rounding is lost.
  - Heads are processed in pairs (2p, 2p+1). qT/kT tiles pack the pair's
    head dims on partitions [0:64] / [64:128]. The score matmuls are ROW
    TILED: two concurrent 64-row matmuls (tile_position auto-derived from
    base partitions) compute both heads' scores in one 512-column pass.
  - The pair's two score outputs go to one [128,1024] 2-bank PSUM tile so
    a single ACT instruction applies exp to both (halves ACT bubbles).
    The exp stream is the pacing engine (~107us total); the PE queue is
    in-order, so every non-attention matmul is CO-EMITTED in small pieces
    inside the ACT-paced attention loops (filler emitted after blocked
    work can never fill a gap):
      * the kt loop is software-pipelined (scores/exp lead the
        attention-value matmuls by SKEW=3 key tiles);
      * v-projection: tiles 0-3 run before attention; tiles 4-7 are
        interleaved one half-group per kt step into pair-0/qc-0;
      * qk-projection for pair p+1 is interleaved piecewise into pair
        p's two phases;
      * output projection for the first query half is interleaved into
        pair-5/qc-1; only the second half's projection is a tail.
  - Lead-in DMA: one wide SBUF tile per tensor, one or two posts each
    (posting costs ~0.6us apiece), need-ordered across the sync and
    gpsimd DGE rings; pair-0 weight slices and qc0 activations first.
  - v is produced in natural layout with an extra all-ones column per head
    (65-col pitch); the attention-value matmul (M=65) accumulates softmax
    denominators for free in its last output row.
  - Normalization runs off the PE critical path: DVE drains both av PSUM
    banks first, a DMA shifts the denominator row to partition 0, fast
    reciprocal, gpsimd partition-broadcast, DVE multiply into outT.
  - y is written back as bf16 (cast to f32 on host).
"""
import numpy as np
import ml_dtypes

import concourse.bass as bass
import concourse.bacc as bacc
import concourse.mybir as mybir
import concourse.tile as tile
from concourse.bass_utils import run_bass_kernel_spmd

F32 = mybir.dt.float32
BF16 = mybir.dt.bfloat16
EXP = mybir.ActivationFunctionType.Exp

B, N, C, H, HD = 8, 1024, 768, 12, 64
P = H // 2              # 6 head pairs
CT = C // 128           # 6 contraction tiles over C
QC = N // 512           # 2 query chunks of 512
KT = N // 128           # 8 key tiles of 128
EC = 2                  # output-projection feature chunks of 384
VP = HD + 1             # 65-col pitch per head in vaug
SKEW = 3                # score/exp lead over av in the kt pipeline
N_CORES = 8

_NC_CACHE = None


class Interleaver:
    """Spreads a list of emission pieces evenly over `slots` call points."""

    def __init__(self, pieces, slots, delay=0):
        self.pieces = list(pieces)
        self.total = len(self.pieces)
        self.slots = slots - delay
        self.delay = delay
        self.done = 0

    def emit(self, slot_idx):
        slot_idx -= self.delay
        if slot_idx < 0:
            return
        target = self.total * (slot_idx + 1) // self.slots
        while self.done < min(target, self.total):
            self.pieces[self.done]()
            self.done += 1

    def finish(self):
        while self.done < self.total:
            self.pieces[self.done]()
            self.done += 1


def _build():
    nc = bacc.Bacc(None, target_bir_lowering=False)

    # all inputs are exact SBUF images ([128, X]) so every DMA moves 128
    # contiguous multi-KB segments (small segments throttle the DMA queues)
    xTi = nc.dram_tensor("xTi", [128, QC * CT * 512], BF16,
                         kind="ExternalInput")
    wqi = nc.dram_tensor("wqi", [128, CT * C], BF16, kind="ExternalInput")
    wki = nc.dram_tensor("wki", [128, CT * C], BF16, kind="ExternalInput")
    wvi = nc.dram_tensor("wvi", [128, CT * C], BF16, kind="ExternalInput")
    wpti = nc.dram_tensor("wpti", [128, CT * C], BF16, kind="ExternalInput")
    bias = nc.dram_tensor("bias", [1, C], F32, kind="ExternalInput")
    y = nc.dram_tensor("y", [N, C], BF16, kind="ExternalOutput")

    from contextlib import ExitStack
    with tile.TileContext(nc) as tc:
        with ExitStack() as ctx:
            pool = lambda name, bufs, **kw: ctx.enter_context(
                tc.tile_pool(name=name, bufs=bufs, **kw))
            vaug_pool = pool("vaug", KT)
            qt_pool = pool("qtp", 4)              # qT pair tiles, 2 pairs
            kt_pool = pool("ktp", 4)
            et_pool = pool("expp", 6)
            avs_pool = pool("avsp", 4)
            iv_pool = pool("ivp", 4)
            bc_pool = pool("bcp", 3)
            ost_pool = pool("ostp", 3)
            out_pool = pool("outp", 2 * CT)
            y_pool = pool("yp", 3)
            cst_pool = pool("cst", 1)
            proj_ps = pool("proj_ps", 2, space="PSUM")
            sc_ps = pool("sc_ps", 2, space="PSUM")
            av_ps = pool("av_ps", 2, space="PSUM")

            # ---- PE warm-up: dummy matmuls bridge the DMA lead-in so the
            # HAM clock gate opens before real work arrives -----------------
            wuf = cst_pool.tile([128, 512], F32, tag="wuf")
            nc.vector.memset(wuf, 0.0)
            wur = cst_pool.tile([128, 512], BF16, tag="wur")
            nc.vector.tensor_copy(wur, wuf)

            for i in range(18):
                wps = proj_ps.tile([128, 512], F32, tag="mmps",
                                   name=f"wu_{i}")
                nc.tensor.matmul(wps, wur[:, 0:128], wur,
                                 start=True, stop=True)

            # ---- loads: SBUF-image tiles, need-ordered wide posts --------
            # wq/wk image layout: [pair-0 block (c-major, 6x128)] ++
            #                     [pairs 1-5 block (c-major, 6x640)]
            # xT image layout:    [qc0 block (c-major, 6x512)] ++ [qc1 block]
            wq_all = cst_pool.tile([128, CT * C], BF16, tag="wq_all")
            wk_all = cst_pool.tile([128, CT * C], BF16, tag="wk_all")
            wv_all = cst_pool.tile([128, CT * C], BF16, tag="wv_all")
            xt_all = cst_pool.tile([128, QC * CT * 512], BF16, tag="xt_all")
            bias_bc = cst_pool.tile([128, C], F32, tag="biasbc")
            wptt = cst_pool.tile([128, CT * C], BF16, tag="wptt")

            P0W = CT * 128      # 768 cols in the pair-0 block
            nc.sync.dma_start(out=wq_all[:, 0:P0W], in_=wqi[:, 0:P0W])
            nc.gpsimd.dma_start(out=wk_all[:, 0:P0W], in_=wki[:, 0:P0W])
            nc.sync.dma_start(out=xt_all[:, 0:CT * 512],
                              in_=xTi[:, 0:CT * 512])
            nc.gpsimd.dma_start(out=wv_all, in_=wvi[:, :])
            nc.sync.dma_start(out=xt_all[:, CT * 512:2 * CT * 512],
                              in_=xTi[:, CT * 512:2 * CT * 512])
            nc.gpsimd.dma_start(out=wq_all[:, P0W:], in_=wqi[:, P0W:])
            nc.gpsimd.dma_start(out=wk_all[:, P0W:], in_=wki[:, P0W:])
            # last in each queue's FIFO: transfers follow the critical loads
            nc.sync.dma_start(out=wptt, in_=wpti[:, :])
            nc.gpsimd.dma_start(out=bias_bc,
                                in_=bias[:, :].to_broadcast([128, C]))

            def wsl(w_all, p, c):   # [128, 128] stationary slice
                if p == 0:
                    return w_all[:, c * 128:(c + 1) * 128]
                base = P0W + c * 640 + (p - 1) * 128
                return w_all[:, base:base + 128]

            def xt(c, qc):      # [128, 512] moving slice, contiguous rows
                base = qc * CT * 512 + c * 512
                return xt_all[:, base:base + 512]

            def xt128(c, tt):   # [128, 128] stationary slice
                base = (tt // 4) * CT * 512 + c * 512 + (tt % 4) * 128
                return xt_all[:, base:base + 128]

            ones12 = cst_pool.tile([128, H], BF16, tag="ones12")
            nc.vector.memset(ones12, 1.0)

            # ---- q/k projection for pair p, as a list of small pieces ----
            def qk_pieces(p, store):
                """Emission pieces (closures); store receives qts/kts."""
                store["q"] = [None] * QC
                store["k"] = [None] * QC
                pieces = []
                for qc in range(QC):
                    for qk in range(2):
                        grp = {}

                        def mm(c, p=p, qc=qc, qk=qk, grp=grp):
                            if c == 0:
                                dst_pool = qt_pool if qk == 0 else kt_pool
                                grp["st"] = dst_pool.tile(
                                    [128, 512], BF16, tag="st",
                                    name=f"st{p}_{qk}_{qc}")
                                grp["ps"] = proj_ps.tile(
                                    [128, 512], F32, tag="mmps",
                                    name=f"pqk{p}_{qk}_{qc}")
                            w_all = wq_all if qk == 0 else wk_all
                            nc.tensor.matmul(
                                grp["ps"], wsl(w_all, p, c), xt(c, qc),
                                start=(c == 0), stop=(c == CT - 1),
                            )

                        def drain(qc=qc, qk=qk, grp=grp, store=store):
                            nc.vector.tensor_copy(grp["st"], grp["ps"])
                            store["q" if qk == 0 else "k"][qc] = grp["st"]

                        for c in range(CT):
                            pieces.append(lambda c=c, mm=mm: mm(c))
                        pieces.append(drain)
                return pieces

            # ---- v_aug[tt] = [v | 1] per head, natural layout ------------
            vaug = [None] * KT

            def vproj_half(tt, half):
                if half == 0:
                    vaug[tt] = vaug_pool.tile([128, H * VP], BF16,
                                              tag="vaug", name=f"vaug{tt}")
                va = vaug[tt]
                pv = proj_ps.tile([128, 384], F32, tag="mmps",
                                  name=f"pv{tt}_{half}")
                for c in range(CT):
                    nc.tensor.matmul(
                        pv, xt128(c, tt),
                        wv_all[:, c * C + half * 384:c * C + (half + 1) * 384],
                        start=(c == 0), stop=(c == CT - 1),
                    )
                dst = bass.AP(tensor=va.tensor,
                              offset=va.offset + half * 6 * VP,
                              ap=[va.ap[0], [VP, 6], [1, HD]])
                nc.vector.tensor_copy(dst, pv)
                if half == 1:
                    ones_ap = bass.AP(tensor=va.tensor, offset=va.offset + HD,
                                      ap=[va.ap[0], [VP, H]])
                    nc.vector.tensor_copy(ones_ap, ones12)

            # ---- output accumulator tiles (c-major, [128, 512] per qc) ---
            outT = [[out_pool.tile([128, 512], BF16, tag="outT",
                                   name=f"outT{i}_{qc}")
                     for qc in range(QC)] for i in range(CT)]

            # ---- output projection pieces --------------------------------
            def proj_pieces(tts):
                pieces = []
                for tt in tts:
                    grp = {}

                    def mm(ec, c, tt=tt, grp=grp):
                        if ec == 0 and c == 0:
                            grp["ysb"] = y_pool.tile([128, C], BF16, tag="y",
                                                     name=f"y{tt}")
                        if c == 0:
                            grp["ps"] = proj_ps.tile([128, 384], F32,
                                                     tag="mmps",
                                                     name=f"py{tt}_{ec}")
                        nc.tensor.matmul(
                            grp["ps"],
                            outT[c][tt // 4][:, (tt % 4) * 128:(tt % 4 + 1) * 128],
                            wptt[:, c * C + ec * 384:c * C + (ec + 1) * 384],
                            start=(c == 0), stop=(c == CT - 1),
                        )

                    def fin(ec, tt=tt, grp=grp):
                        nc.vector.tensor_add(
                            grp["ysb"][:, ec * 384:(ec + 1) * 384], grp["ps"],
                            bias_bc[:, ec * 384:(ec + 1) * 384])
                        if ec == EC - 1:
                            nc.sync.dma_start(
                                out=y[tt * 128:(tt + 1) * 128, :],
                                in_=grp["ysb"])

                    for ec in range(EC):
                        for c in range(CT):
                            pieces.append(lambda ec=ec, c=c, mm=mm: mm(ec, c))
                        pieces.append(lambda ec=ec, fin=fin: fin(ec))
                return pieces

            # ---- attention building blocks -------------------------------
            def score_exp(p, qc, qts, kts, kt):
                ps_s = sc_ps.tile([128, 1024], F32, tag="sc",
                                  name=f"sc{p}_{qc}_{kt}")
                klhs = kts[kt // 4][:, (kt % 4) * 128:(kt % 4 + 1) * 128]
                nc.tensor.matmul(
                    ps_s[:, 0:512], klhs[0:64, :], qts[qc][0:64, :],
                    start=True, stop=True,
                )
                nc.tensor.matmul(
                    ps_s[:, 512:1024], klhs[64:128, :], qts[qc][64:128, :],
                    start=True, stop=True,
                )
                et = et_pool.tile([128, 1024], BF16, tag="exp",
                                  name=f"exp{p}_{qc}_{kt}")
                nc.scalar.activation(out=et, in_=ps_s, func=EXP)
                return et

            def av_step(p, av0, av1, et, kt):
                h0, h1 = 2 * p, 2 * p + 1
                nc.tensor.matmul(
                    av0, vaug[kt][:, h0 * VP:h0 * VP + VP], et[:, 0:512],
                    start=(kt == 0), stop=(kt == KT - 1),
                )
                nc.tensor.matmul(
                    av1, vaug[kt][:, h1 * VP:h1 * VP + VP], et[:, 512:1024],
                    start=(kt == 0), stop=(kt == KT - 1),
                )

            def attn(p, qc, qts, kts, inter=None):
                """Software-pipelined kt loop: scores/exp lead av by SKEW.
                inter: Interleaver whose pieces spread across the loop.
                Returns tail pieces (last avs + drains + normalization) for
                the caller to chain into the NEXT phase's interleave."""
                h0, h1 = 2 * p, 2 * p + 1
                av0 = av_ps.tile([VP, 512], F32, tag="av", name=f"av{h0}_{qc}")
                av1 = av_ps.tile([VP, 512], F32, tag="av", name=f"av{h1}_{qc}")
                ets = [None] * KT
                for kt in range(KT):
                    ets[kt] = score_exp(p, qc, qts, kts, kt)
                    if inter is not None:
                        inter.emit(kt)
                    if kt >= SKEW:
                        av_step(p, av0, av1, ets[kt - SKEW], kt - SKEW)
                        ets[kt - SKEW] = None
                if inter is not None:
                    inter.finish()
                tail = []
                for kt in range(KT - SKEW, KT):
                    tail.append(lambda kt=kt: av_step(p, av0, av1, ets[kt], kt))

                def drain(hi, av):
                    avs = avs_pool.tile([VP, 512], F32, tag="avs",
                                        name=f"avs{2 * p + hi}_{qc}")
                    avss[hi] = avs
                    nc.vector.tensor_copy(avs, av)

                avss = [None, None]
                tail.append(lambda: drain(0, av0))
                tail.append(lambda: drain(1, av1))

                def norm(hi):
                    h = 2 * p + hi
                    avs = avss[hi]
                    # row 64 = softmax denominators; shift to partition 0
                    sm0 = iv_pool.tile([1, 512], F32, tag="sm0",
                                       name=f"sm0{h}_{qc}")
                    nc.sync.dma_start(out=sm0, in_=avs[HD:VP, :])
                    iv0 = iv_pool.tile([1, 512], F32, tag="iv0",
                                       name=f"iv0{h}_{qc}")
                    nc.vector.reciprocal_approx_fast(out=iv0, in_=sm0)
                    bc = bc_pool.tile([64, 512], F32, tag="bc",
                                      name=f"bc{h}_{qc}")
                    nc.gpsimd.partition_broadcast(bc, iv0)
                    if hi == 0:
                        nc.vector.tensor_mul(
                            outT[p][qc][0:64, :], avs[0:HD, :], bc)
                    else:
                        ost = ost_pool.tile([64, 512], BF16, tag="ost",
                                            name=f"ost{h}_{qc}")
                        nc.vector.tensor_mul(ost, avs[0:HD, :], bc)
                        nc.sync.dma_start(out=outT[p][qc][64:128, :],
                                          in_=ost)

                tail.append(lambda: norm(0))
                tail.append(lambda: norm(1))
                return tail

            # ---- pipeline ------------------------------------------------
            # pair-0 qc0 q/k groups run first (their inputs arrive first);
            # everything else (v-projection, pair-0 qc1 q/k, next-pair q/k,
            # phase tails, projection pieces) is chained through the
            # ACT-paced attention loops' interleavers.
            st0 = {}
            qk0 = qk_pieces(0, st0)
            for piece in qk0[:14]:       # (q,qc0) and (k,qc0) groups
                piece()
            qts, kts = st0["q"], st0["k"]

            vpieces = [lambda tt=tt, half=half: vproj_half(tt, half)
                       for tt in range(KT) for half in range(2)]
            # qc1 q/k groups first (kts[1] is referenced from kt=4 on), and
            # one slot early so vaug[kt] exists before its av matmul
            tail = attn(0, 0, qts, kts,
                        inter=Interleaver(qk0[14:] + vpieces, KT - 1))

            st_nxt = {}
            tail = attn(0, 1, qts, kts,
                        inter=Interleaver(tail + qk_pieces(1, st_nxt), KT))

            for p in range(1, P):
                qts, kts = st_nxt["q"], st_nxt["k"]
                last = (p == P - 1)
                if not last:
                    st_nxt = {}
                    pieces = qk_pieces(p + 1, st_nxt)
                    tail = attn(p, 0, qts, kts,
                                inter=Interleaver(tail + pieces[:14], KT))
                    tail = attn(p, 1, qts, kts,
                                inter=Interleaver(tail + pieces[14:], KT))
                else:
                    tail = attn(p, 0, qts, kts, inter=Interleaver(tail, KT))
                    # first query half's projection rides pair-5/qc-1;
                    # one slot late so outT[5][0]'s norm completes
                    tail = attn(p, 1, qts, kts,
                                inter=Interleaver(tail + proj_pieces(range(4)),
                                                  KT, delay=1))

            # tail: last phase's avs/norm, then the second half's projection
            for piece in tail:
                piece()
            for piece in proj_pieces(range(4, KT)):
                piece()

    nc.finalize()
    return nc


def _get_nc():
    global _NC_CACHE
    if _NC_CACHE is None:
        _NC_CACHE = _build()
    return _NC_CACHE


def _host_prep(x, Wqkv, Wproj, bproj, Aq, Bq, Av, Bv):
    """Fold LoRA + score scale into the weights; lay out and cast to bf16."""
    bf16 = ml_dtypes.bfloat16
    W = Wqkv.astype(np.float64)
    Wq = W[0:C].reshape(H, HD, C)
    Wk = W[C:2 * C].reshape(H, HD, C)
    Wv_ = W[2 * C:3 * C].reshape(H, HD, C)
    ABq = Aq.astype(np.float64) @ Bq.astype(np.float64)   # [HD, HD]
    ABv = Av.astype(np.float64) @ Bv.astype(np.float64)
    Wq = Wq + np.einsum('ed,hec->hdc', ABq, Wq)           # (I+AB).T @ Wq per head
    Wv_ = Wv_ + np.einsum('ed,hec->hdc', ABv, Wv_)
    Wq = Wq * (HD ** -0.5)                                # fold score scale

    # wq/wk[c] = [K=c-rows(128), 768 = 12 heads x 64 dims, head-major]
    wq_ = np.empty((CT, 128, C), np.float32)
    wk_ = np.empty((CT, 128, C), np.float32)
    for h in range(H):
        for c in range(CT):
            cs = slice(c * 128, (c + 1) * 128)
            wq_[c, :, h * 64:(h + 1) * 64] = Wq[h][:, cs].T.astype(np.float32)
            wk_[c, :, h * 64:(h + 1) * 64] = Wk[h][:, cs].T.astype(np.float32)

    # wv[c] = [K=c-rows(128), all 768 v output features]
    WvT = Wv_.reshape(C, C).T.astype(np.float32)          # [c_in, v_out]
    wv_ = WvT.reshape(CT, 128, C)

    # wpt[c] = Wproj.T c-tiles: [K=c(128), e(768)]
    WpT = Wproj.astype(np.float32).T                      # [c, e]
    wpt_ = WpT.reshape(CT, 128, C)

    bias_ = bproj.astype(np.float32).reshape(1, C)

    def qk_image(w):
        # [CT,128,C] -> [128, pair0 block (CT x 128) ++ rest (CT x 640)]
        t = w.transpose(1, 0, 2)                          # [128, CT, C]
        blk0 = t[:, :, 0:128].reshape(128, CT * 128)
        blk1 = t[:, :, 128:C].reshape(128, CT * 640)
        return np.ascontiguousarray(
            np.concatenate([blk0, blk1], axis=1)).astype(bf16)

    def cmaj_image(w):
        # [CT,128,X] -> [128, CT * X]
        return np.ascontiguousarray(
            w.transpose(1, 0, 2).reshape(128, -1)).astype(bf16)

    wq16 = qk_image(wq_)
    wk16 = qk_image(wk_)
    wv16 = cmaj_image(wv_)
    wpt16 = cmaj_image(wpt_)

    per_core = []
    for b in range(B):
        # x image: [128, qc-major ++ c-major ++ 512 tokens]
        xTb = x[b].astype(np.float32).T.reshape(CT, 128, QC, 512)
        xTb = np.ascontiguousarray(
            xTb.transpose(1, 2, 0, 3).reshape(128, -1)).astype(bf16)
        per_core.append({"xTi": xTb, "wqi": wq16, "wki": wk16, "wvi": wv16,
                         "wpti": wpt16, "bias": bias_})
    return per_core


def kernel(x, Wqkv, Wproj, bproj, Aq, Bq, Av, Bv, _trace=False):
    x = np.asarray(x)
    in_maps = _host_prep(np.asarray(x), np.asarray(Wqkv), np.asarray(Wproj),
                         np.asarray(bproj), np.asarray(Aq), np.asarray(Bq),
                         np.asarray(Av), np.asarray(Bv))
    nc = _get_nc()
    res = run_bass_kernel_spmd(nc, in_maps, core_ids=list(range(N_CORES)),
                               trace=_trace)
    out = np.stack([res.results[b]["y"] for b in range(B)], axis=0)
    if _trace:
        kernel._last_result = res
    return out.astype(np.float32)
